# revision 14
# baseline (speedup 1.0000x reference)
"""MQA attention kernel for 8 Trainium2 NeuronCores.

Sharding: tensor-parallel over the 32 query heads (4 heads per core), shared
K/V head replicated. After per-head attention, the per-head outputs
(attnT [head_dim, S]) are AllGathered; the dense projection is column-sharded
(each core computes 512 of the 4096 output features for all tokens), so the
host-side unshard is a concat, with no cross-core reduction.

All matmuls run in bf16 with fp32 PSUM accumulation.

Device layouts (partition dim first):
  hsT   [128 h_in, 32 h_out, 2048 s]   transposed hidden states (replicated)
  wq    [128 h_in, 32 h_out, 512  m]   wq_w.T slice for this core's 4 heads
  wk/wv [128 h_in, 32 h_out, 128  d]   shared K/V projections (replicated)
  dw    [128 hd_in, 32 head, 512  o]   dense_w.T column slice (this core's o)
  cosT/sinT [128 d, 2048 s]
  mask  [128 r, 4 delta, 512 c]        causal 0/1 tiles for diagonal blocks

Per-core compute:
  qT/kT projections -> RoPE -> scoresT [sk,sq] = kT.T @ qT -> exp (scale folded
  into ACT) -> causal mask multiply -> column sums via ones-matmul ->
  attnT [d,sq] = v.T-free accumulate -> normalize -> AllGather -> dense.
"""

import numpy as np
import ml_dtypes

S = 2048
H = 4096
NH = 32
HD = 128
NCORES = 8
HPC = NH // NCORES  # 4 heads per core
OSH = H // NCORES  # 512 output features per core
SB = 512  # sequence block for matmul free dim
NSB = S // SB  # 4
NKB = S // HD  # 16 sk blocks of 128
SCALE = 1.0 / float(HD) ** 0.5

BF16 = ml_dtypes.bfloat16


def _build(num_devices=NCORES):
    import concourse.bass as bass
    import concourse.tile as tile
    import concourse.mybir as mybir
    from concourse import bacc
    from concourse.bass import ts
    from concourse.masks import make_identity

    f32 = mybir.dt.float32
    bf16 = mybir.dt.bfloat16

    nc = bacc.Bacc("TRN2", target_bir_lowering=False, debug=False,
                   num_devices=num_devices)

    hsT = nc.dram_tensor("hsT", [128, 32, S], bf16, kind="ExternalInput").ap()
    wq = nc.dram_tensor("wq", [128, 32, OSH], bf16, kind="ExternalInput").ap()
    wk = nc.dram_tensor("wk", [128, 32, HD], bf16, kind="ExternalInput").ap()
    wv = nc.dram_tensor("wv", [128, 32, HD], bf16, kind="ExternalInput").ap()
    dw = nc.dram_tensor("dw", [128, 32, OSH], bf16, kind="ExternalInput").ap()
    cosT = nc.dram_tensor("cosT", [128, S], bf16, kind="ExternalInput").ap()
    sinT = nc.dram_tensor("sinT", [128, S], bf16, kind="ExternalInput").ap()
    maskM = nc.dram_tensor("maskM", [128, 4, SB], bf16, kind="ExternalInput").ap()
    rotm = nc.dram_tensor("rotm", [128, 128], bf16, kind="ExternalInput").ap()

    out_sl = nc.dram_tensor("out_sl", [S, OSH], f32, kind="ExternalOutput").ap()
    k_out = nc.dram_tensor("k_out", [128, S], f32, kind="ExternalOutput").ap()
    v_out = nc.dram_tensor("v_out", [128, S], f32, kind="ExternalOutput").ap()

    RG = [list(range(num_devices))]

    from contextlib import ExitStack

    with tile.TileContext(nc) as tc, ExitStack() as ctx:
        const = ctx.enter_context(tc.tile_pool(name="const", bufs=1))
        bigw = ctx.enter_context(tc.tile_pool(name="bigw", bufs=1))
        hsp = ctx.enter_context(tc.tile_pool(name="hsp", bufs=2))
        persist = ctx.enter_context(tc.tile_pool(name="persist", bufs=1))
        work = ctx.enter_context(tc.tile_pool(name="work", bufs=2))
        expp = ctx.enter_context(tc.tile_pool(name="expp", bufs=4))
        abp = ctx.enter_context(tc.tile_pool(name="abp", bufs=4))
        psum_acc = ctx.enter_context(
            tc.tile_pool(name="psum_acc", bufs=3, space="PSUM"))
        psum_sc = ctx.enter_context(
            tc.tile_pool(name="psum_sc", bufs=4, space="PSUM"))
        psum_sm = ctx.enter_context(
            tc.tile_pool(name="psum_sm", bufs=1, space="PSUM"))
        dram = ctx.enter_context(tc.tile_pool(name="dram", bufs=16, space="DRAM"))

        # ---- constants ----
        # DMA emission order = approximate issue order; load in the order
        # compute needs the data: wk/wv -> first hs block -> wq -> rest.
        wk_sb = const.tile([128, 32, HD], bf16)
        nc.sync.dma_start(wk_sb[:], wk[:])
        wv_sb = const.tile([128, 32, HD], bf16)
        nc.sync.dma_start(wv_sb[:], wv[:])

        hs0_sb = hsp.tile([128, 32, SB], bf16, tag="hs")
        for hc in range(8):
            nc.sync.dma_start(hs0_sb[:, ts(hc, 4), :],
                              hsT[:, ts(hc, 4), ts(0, SB)])

        # big weights share one slot: wq used in phase 1, dw in phase 3
        wq_sb = bigw.tile([128, 32, OSH], bf16, tag="bigw")
        nc.sync.dma_start(wq_sb[:], wq[:])

        cos_sb = const.tile([128, S], bf16)
        nc.sync.dma_start(cos_sb[:], cosT[:])
        sin_sb = const.tile([128, S], bf16)
        nc.sync.dma_start(sin_sb[:], sinT[:])
        mask_sb = const.tile([128, 4, SB], bf16)
        nc.sync.dma_start(mask_sb[:], maskM[:])
        ones_sb = const.tile([128, 128], bf16)
        nc.gpsimd.memset(ones_sb[:], 1.0)
        ident = const.tile([128, 128], bf16)
        make_identity(nc, ident[:])
        rot_sb = const.tile([128, 128], bf16)
        nc.sync.dma_start(rot_sb[:], rotm[:])

        # persistent activations
        qr_sb = persist.tile([128, HPC, S], bf16)  # RoPE'd qT per local head
        kr_sb = persist.tile([128, S], bf16)       # RoPE'd kT
        vt_sb = persist.tile([128, S], bf16)       # vT
        vn_sb = persist.tile([128, NKB, HD], bf16)  # v natural [sk, d] blocks

        def rope(dst, src, js):
            # dst/src [128, 512] bf16. rotate_half(x) done as a PE matmul
            # with the constant +/-1 rotation matrix (DVE lanes cannot read
            # across partitions), then two aligned muls + add.
            c = cos_sb[:, ts(js, SB)]
            si = sin_sb[:, ts(js, SB)]
            pr = psum_acc.tile([128, SB], f32, tag="acc")
            nc.tensor.matmul(pr[:], rot_sb[:], src[:], start=True, stop=True)
            rb = work.tile([128, SB], bf16, tag="rope_b")
            nc.any.tensor_copy(rb[:], pr[:])
            a = work.tile([128, SB], bf16, tag="rope_a")
            nc.vector.tensor_mul(a[:], src[:], c)
            nc.vector.tensor_mul(rb[:], rb[:], si)
            nc.vector.tensor_add(dst[:], a[:], rb[:])

        # ---- phase 1: QKV projections + RoPE ----
        # All 6 projection groups for an s-block are emitted first; the RoPE
        # matmuls (which depend on cross-engine PSUM->SBUF copies) follow, so
        # the PE never head-of-line blocks on a copy.
        for j in range(NSB):
            if j == 0:
                hs_sb = hs0_sb
            else:
                hs_sb = hsp.tile([128, 32, SB], bf16, tag="hs")
                for hc in range(8):
                    nc.sync.dma_start(hs_sb[:, ts(hc, 4), :],
                                      hsT[:, ts(hc, 4), ts(j, SB)])

            # k projection
            pk = psum_acc.tile([128, SB], f32, tag="acc")
            for ho in range(32):
                nc.tensor.matmul(pk[:], wk_sb[:, ho, :], hs_sb[:, ho, :],
                                 start=(ho == 0), stop=(ho == 31))
            kraw = work.tile([128, SB], bf16, tag="kraw")
            nc.any.tensor_copy(kraw[:], pk[:])

            # v projection
            pv = psum_acc.tile([128, SB], f32, tag="acc")
            for ho in range(32):
                nc.tensor.matmul(pv[:], wv_sb[:, ho, :], hs_sb[:, ho, :],
                                 start=(ho == 0), stop=(ho == 31))
            v32 = work.tile([128, SB], f32, tag="k32")
            nc.any.tensor_copy(v32[:], pv[:])
            nc.sync.dma_start(v_out[:, ts(j, SB)], v32[:])
            nc.any.tensor_copy(vt_sb[:, ts(j, SB)], pv[:])

            # q projections (4 local heads)
            qraws = []
            for t in range(HPC):
                pq = psum_acc.tile([128, SB], f32, tag="acc")
                for ho in range(32):
                    nc.tensor.matmul(pq[:], wq_sb[:, ho, ts(t, HD)],
                                     hs_sb[:, ho, :],
                                     start=(ho == 0), stop=(ho == 31))
                qraw = work.tile([128, SB], bf16, tag="qraw", bufs=5,
                                 name=f"qraw_{j}_{t}")
                nc.any.tensor_copy(qraw[:], pq[:])
                qraws.append(qraw)

            # RoPE (trailing: PSUM copies had time to complete)
            rope(kr_sb[:, ts(j, SB)], kraw, j)
            k32 = work.tile([128, SB], f32, tag="k32")
            nc.any.tensor_copy(k32[:], kr_sb[:, ts(j, SB)])
            nc.sync.dma_start(k_out[:, ts(j, SB)], k32[:])
            for t in range(HPC):
                rope(qr_sb[:, t, ts(j, SB)], qraws[t], j)

            # v transpose for this s-block (4 sk blocks of 128)
            for ii in range(4):
                i = 4 * j + ii
                pt = psum_acc.tile([128, HD], bf16, tag="acc",
                                   padded_shape=[128, SB])
                nc.tensor.transpose(pt[:], vt_sb[:, ts(i, HD)], ident[:])
                nc.any.tensor_copy(vn_sb[:, i, :], pt[:])

        # ---- phase 2: attention per head ----
        # AllGather bounce buffers, one per (local head, seq half)
        agin = [[dram.tile([128, 2 * SB], bf16, name=f"agin_{t}_{hf}")
                 for hf in range(2)] for t in range(HPC)]
        agout = [[dram.tile([NCORES * 128, 2 * SB], bf16,
                            addr_space="Shared", name=f"agout_{t}_{hf}")
                  for hf in range(2)] for t in range(HPC)]

        def attention(t, j):
            nblk = 4 * j + 4  # causal sk blocks
            pat = psum_acc.tile([128, SB], f32, tag="acc",
                                name=f"pat_{t}_{j}")
            psm = psum_sm.tile([1, SB], f32, tag="sm", name=f"psm_{t}_{j}")
            for i in range(nblk):
                psc = psum_sc.tile([128, SB], f32, tag="sc",
                                   name=f"psc_{t}_{j}_{i}")
                nc.tensor.matmul(psc[:], kr_sb[:, ts(i, HD)],
                                 qr_sb[:, t, ts(j, SB)],
                                 start=True, stop=True)
                est = expp.tile([128, SB], bf16, tag="est",
                                name=f"est_{t}_{j}_{i}")
                nc.scalar.activation(est[:], psc[:],
                                     mybir.ActivationFunctionType.Exp,
                                     scale=SCALE)
                if i >= 4 * j:  # diagonal block: causal 0/1 mask
                    nc.vector.tensor_mul(est[:], est[:],
                                         mask_sb[:, i - 4 * j, :])
                nc.tensor.matmul(psm[:], ones_sb[:, 0:1], est[:],
                                 start=(i == 0), stop=(i == nblk - 1))
                nc.tensor.matmul(pat[:], vn_sb[:, i, :], est[:],
                                 start=(i == 0), stop=(i == nblk - 1))
            rec = work.tile([1, SB], f32, tag="rec", name=f"rec_{t}_{j}")
            nc.vector.reciprocal(rec[:], psm[:])
            recb = work.tile([1, SB], bf16, tag="recb", name=f"recb_{t}_{j}")
            nc.any.tensor_copy(recb[:], rec[:])
            pbc = psum_acc.tile([128, SB], f32, tag="acc",
                                name=f"pbc_{t}_{j}")
            nc.tensor.matmul(pbc[:], ones_sb[0:1, :], recb[:],
                             start=True, stop=True)
            atf = work.tile([128, SB], f32, tag="atf", name=f"atf_{t}_{j}")
            nc.any.tensor_copy(atf[:], pat[:])
            atb = work.tile([128, SB], bf16, tag="atb", name=f"atb_{t}_{j}")
            nc.vector.tensor_mul(atb[:], atf[:], pbc[:])
            nc.sync.dma_start(agin[t][j // 2][:, ts(j % 2, SB)], atb[:])

        def allgather(t, hf):
            if num_devices == NCORES:
                nc.gpsimd.collective_compute(
                    "AllGather", mybir.AluOpType.bypass, replica_groups=RG,
                    ins=[agin[t][hf].opt()], outs=[agout[t][hf].opt()])
            else:
                # single-core sim variant: stand-in copy keeps the dense
                # dependencies representative.
                nc.sync.dma_start(agout[t][hf][0:128, :], agin[t][hf][:])

        def dense_chunk(hf, sqc):
            # one 128-row sq chunk of this half: out rows q16*128..
            q16 = hf * 8 + sqc
            pd = psum_acc.tile([128, OSH], f32, tag="acc",
                               name=f"pd_{hf}_{sqc}")
            first = True
            for t in range(HPC):
                agv = agout[t][hf].rearrange("(c p) s -> p c s", p=128)
                ab = abp.tile([128, NCORES, HD], bf16, tag="ab",
                              name=f"ab_{hf}_{sqc}_{t}")
                nc.sync.dma_start(ab[:], agv[:, :, ts(sqc, HD)])
                for c in range(NCORES):
                    gh = HPC * c + t
                    nc.tensor.matmul(pd[:], ab[:, c, :], dw_sb[:, gh, :],
                                     start=first,
                                     stop=(t == HPC - 1 and c == NCORES - 1))
                    first = False
            of = work.tile([128, OSH], f32, tag="of", name=f"of_{hf}_{sqc}")
            nc.any.tensor_copy(of[:], pd[:])
            nc.sync.dma_start(out_sl[ts(q16, 128), :], of[:])

        # half 0 attention for all heads, AG per head as soon as ready
        for t in range(HPC):
            attention(t, 0)
            attention(t, 1)
            allgather(t, 0)

        # dense weights load (wq slot is free now; overlaps attention)
        dw_sb = bigw.tile([128, 32, OSH], bf16, tag="bigw")
        nc.sync.dma_start(dw_sb[:], dw[:])

        # half 1 attention with dense half-0 chunks interleaved
        for t in range(HPC):
            attention(t, 2)
            attention(t, 3)
            allgather(t, 1)
            dense_chunk(0, 2 * t)
            dense_chunk(0, 2 * t + 1)

        # dense half 1
        for sqc in range(8):
            dense_chunk(1, sqc)

    nc.compile()
    return nc


_BUILT = None


def _get_nc():
    global _BUILT
    if _BUILT is None:
        _BUILT = _build()
    return _BUILT


def _prep_inputs(hidden_states, cos, sin, wq_w, wk_w, wv_w, dense_w):
    """Host-side shard prep: transposes + bf16 casts + per-core slices."""
    hs = np.asarray(hidden_states, np.float32).reshape(S, H)
    hsT = np.ascontiguousarray(hs.T).reshape(32, 128, S).transpose(1, 0, 2)
    hsT = np.ascontiguousarray(hsT).astype(BF16)

    def wslice(wT, lo, hi):
        # wT [H, m] -> [128, 32, m] partition-major
        w = np.ascontiguousarray(wT[:, lo:hi]).reshape(32, 128, hi - lo)
        return np.ascontiguousarray(w.transpose(1, 0, 2)).astype(BF16)

    wqT = np.asarray(wq_w, np.float32).T  # [H, H]
    wkT = np.asarray(wk_w, np.float32).T  # [H, HD]
    wvT = np.asarray(wv_w, np.float32).T
    dwT = np.asarray(dense_w, np.float32).T  # [H(hd), H(o)]

    wk_l = wslice(wkT, 0, HD)
    wv_l = wslice(wvT, 0, HD)
    cosT = np.ascontiguousarray(
        np.asarray(cos, np.float32).reshape(S, HD).T).astype(BF16)
    sinT = np.ascontiguousarray(
        np.asarray(sin, np.float32).reshape(S, HD).T).astype(BF16)
    # mask tiles: M[r',c] = 1 if r' <= c (allowed), [512,512] -> [128,4,512]
    r = np.arange(SB)
    M = (r[:, None] <= r[None, :]).astype(BF16)
    maskM = np.ascontiguousarray(
        M.reshape(4, 128, SB).transpose(1, 0, 2))
    # rotation matrix: (R @ q)[d] = -q[d+64] (d<64), q[d-64] (d>=64);
    # device matmul computes lhsT.T @ rhs, so feed R.T.
    R = np.zeros((128, 128), np.float32)
    for d in range(64):
        R[d, 64 + d] = -1.0
        R[64 + d, d] = 1.0
    rotm = np.ascontiguousarray(R.T).astype(BF16)

    in_maps = []
    for c in range(NCORES):
        in_maps.append({
            "hsT": hsT,
            "wq": wslice(wqT, OSH * c, OSH * (c + 1)),
            "wk": wk_l,
            "wv": wv_l,
            "dw": wslice(dwT, OSH * c, OSH * (c + 1)),
            "cosT": cosT,
            "sinT": sinT,
            "maskM": maskM,
            "rotm": rotm,
        })
    return in_maps


_RUNNER = None


def _get_runner():
    """Build (once) a jitted shard_map callable over the 8 cores, mirroring
    concourse.bass2jax.run_bass_via_pjrt's multi-core branch, so repeated
    calls reuse the compiled NEFF and we can time executions."""
    global _RUNNER
    if _RUNNER is not None:
        return _RUNNER
    import jax
    import concourse.mybir as mybir
    from jax.sharding import Mesh, PartitionSpec
    from jax.experimental.shard_map import shard_map
    from concourse.bass2jax import (_bass_exec_p, install_neuronx_cc_hook,
                                    partition_id_tensor)

    nc = _get_nc()
    install_neuronx_cc_hook()
    partition_name = (nc.partition_id_tensor.name
                      if nc.partition_id_tensor else None)

    in_names, out_names, out_avals, zero_outs = [], [], [], []
    for alloc in nc.m.functions[0].allocations:
        if not isinstance(alloc, mybir.MemoryLocationSet):
            continue
        name = alloc.memorylocations[0].name
        if alloc.kind == "ExternalInput":
            if name != partition_name:
                in_names.append(name)
        elif alloc.kind == "ExternalOutput":
            out_names.append(name)
            shape = tuple(alloc.tensor_shape)
            dtype = mybir.dt.np(alloc.dtype)
            out_avals.append(jax.core.ShapedArray(shape, dtype))
            zero_outs.append(np.zeros(shape, dtype))
    n_params = len(in_names)
    n_outs = len(out_avals)
    all_in_names = list(in_names) + list(out_names)
    if partition_name is not None:
        all_in_names.append(partition_name)

    def _body(*args):
        operands = list(args)
        if partition_name is not None:
            operands.append(partition_id_tensor())
        outs = _bass_exec_p.bind(
            *operands,
            out_avals=tuple(out_avals),
            in_names=tuple(all_in_names),
            out_names=tuple(out_names),
            lowering_input_output_aliases=(),
            sim_require_finite=True,
            sim_require_nnan=True,
            nc=nc,
        )
        return tuple(outs)

    devices = jax.devices()[:NCORES]
    mesh = Mesh(np.asarray(devices), ("core",))
    in_specs = (PartitionSpec("core"),) * (n_params + n_outs)
    out_specs = (PartitionSpec("core"),) * n_outs
    donate = tuple(range(n_params, n_params + n_outs))
    fn = jax.jit(
        shard_map(_body, mesh=mesh, in_specs=in_specs, out_specs=out_specs,
                  check_rep=False),
        donate_argnums=donate, keep_unused=True)
    _RUNNER = dict(fn=fn, mesh=mesh, in_names=in_names, out_names=out_names,
                   out_avals=out_avals, zero_outs=zero_outs,
                   n_params=n_params)
    return _RUNNER


def _concat_inputs(in_maps):
    r = _get_runner()
    return [np.concatenate([np.asarray(in_maps[c][name])
                            for c in range(NCORES)], axis=0)
            for name in r["in_names"]]


def _zero_bufs():
    r = _get_runner()
    return [np.zeros((NCORES * z.shape[0], *z.shape[1:]), z.dtype)
            for z in r["zero_outs"]]


def _run(in_maps):
    r = _get_runner()
    out_arrs = r["fn"](*_concat_inputs(in_maps), *_zero_bufs())
    res = []
    for c in range(NCORES):
        res.append({name: np.asarray(out_arrs[i]).reshape(
            NCORES, *r["out_avals"][i].shape)[c]
            for i, name in enumerate(r["out_names"])})
    return res


def kernel(hidden_states, cos, sin, wq_w, wk_w, wv_w, dense_w):
    in_maps = _prep_inputs(hidden_states, cos, sin, wq_w, wk_w, wv_w, dense_w)
    res = _run(in_maps)
    out = np.concatenate([res[c]["out_sl"] for c in range(NCORES)], axis=1)
    out = out.reshape(1, S, H)
    key = res[0]["k_out"].T.reshape(1, 1, S, HD)
    value = res[0]["v_out"].T.reshape(1, 1, S, HD)
    return (out, key, value)


def _make_chained_fn(K):
    """jit fn that executes the NEFF K times back-to-back (separate zero
    buffers per execution) so per-dispatch axon overhead amortizes."""
    import jax
    from jax.sharding import PartitionSpec
    from jax.experimental.shard_map import shard_map
    from concourse.bass2jax import _bass_exec_p, partition_id_tensor

    r = _get_runner()
    nc = _get_nc()
    partition_name = (nc.partition_id_tensor.name
                      if nc.partition_id_tensor else None)
    out_names = r["out_names"]
    out_avals = r["out_avals"]
    in_names = r["in_names"]
    n_params = r["n_params"]
    n_outs = len(out_names)
    all_in_names = list(in_names) + list(out_names)
    if partition_name is not None:
        all_in_names.append(partition_name)

    def _body(*args):
        ins = list(args[:n_params])
        outs_all = []
        for k in range(K):
            zeros_k = list(args[n_params + k * n_outs:
                                n_params + (k + 1) * n_outs])
            operands = ins + zeros_k
            if partition_name is not None:
                operands.append(partition_id_tensor())
            outs = _bass_exec_p.bind(
                *operands,
                out_avals=tuple(out_avals),
                in_names=tuple(all_in_names),
                out_names=tuple(out_names),
                lowering_input_output_aliases=(),
                sim_require_finite=True,
                sim_require_nnan=True,
                nc=nc,
            )
            outs_all.extend(outs)
        return tuple(outs_all)

    mesh = r["mesh"]
    nin = n_params + K * n_outs
    fn = jax.jit(
        shard_map(_body, mesh=mesh,
                  in_specs=(PartitionSpec("core"),) * nin,
                  out_specs=(PartitionSpec("core"),) * (K * n_outs),
                  check_rep=False),
        donate_argnums=tuple(range(n_params, nin)), keep_unused=True)
    return fn


# revision 19
# speedup vs baseline: 1.5782x; 1.5782x over previous
"""MQA attention kernel for 8 Trainium2 NeuronCores.

Sharding: tensor-parallel over the 32 query heads (4 heads per core), shared
K/V head replicated. After per-head attention, the per-head outputs
(attnT [head_dim, S]) are AllGathered; the dense projection is column-sharded
(each core computes 512 of the 4096 output features for all tokens), so the
host-side unshard is a concat, with no cross-core reduction.

All matmuls run in bf16 with fp32 PSUM accumulation.

Device layouts (partition dim first):
  hsT   [128 h_in, 32 h_out, 2048 s]   transposed hidden states (replicated)
  wq    [128 h_in, 32 h_out, 512  m]   wq_w.T slice for this core's 4 heads
  wk/wv [128 h_in, 32 h_out, 128  d]   shared K/V projections (replicated)
  dw    [128 hd_in, 32 head, 512  o]   dense_w.T column slice (this core's o)
  cosT/sinT [128 d, 2048 s]
  mask  [128 r, 4 delta, 512 c]        causal 0/1 tiles for diagonal blocks

Per-core compute:
  qT/kT projections -> RoPE -> scoresT [sk,sq] = kT.T @ qT -> exp (scale folded
  into ACT) -> causal mask multiply -> column sums via ones-matmul ->
  attnT [d,sq] = v.T-free accumulate -> normalize -> AllGather -> dense.
"""

import numpy as np
import ml_dtypes

S = 2048
H = 4096
NH = 32
HD = 128
NCORES = 8
HPC = NH // NCORES  # 4 heads per core
OSH = H // NCORES  # 512 output features per core
SB = 512  # sequence block for matmul free dim
NSB = S // SB  # 4
NKB = S // HD  # 16 sk blocks of 128
SCALE = 1.0 / float(HD) ** 0.5

BF16 = ml_dtypes.bfloat16


def _build(num_devices=NCORES):
    import concourse.bass as bass
    import concourse.tile as tile
    import concourse.mybir as mybir
    from concourse import bacc
    from concourse.bass import ts
    from concourse.masks import make_identity

    f32 = mybir.dt.float32
    bf16 = mybir.dt.bfloat16

    nc = bacc.Bacc("TRN2", target_bir_lowering=False, debug=False,
                   num_devices=num_devices)

    hsT = nc.dram_tensor("hsT", [128, 32, S], bf16, kind="ExternalInput").ap()
    wq = nc.dram_tensor("wq", [128, 32, OSH], bf16, kind="ExternalInput").ap()
    wk = nc.dram_tensor("wk", [128, 32, HD], bf16, kind="ExternalInput").ap()
    wv = nc.dram_tensor("wv", [128, 32, HD], bf16, kind="ExternalInput").ap()
    dw = nc.dram_tensor("dw", [128, 32, OSH], bf16, kind="ExternalInput").ap()
    cosT = nc.dram_tensor("cosT", [128, S], bf16, kind="ExternalInput").ap()
    sinT = nc.dram_tensor("sinT", [128, S], bf16, kind="ExternalInput").ap()
    maskM = nc.dram_tensor("maskM", [128, 4, SB], bf16, kind="ExternalInput").ap()
    rotm = nc.dram_tensor("rotm", [128, 128], bf16, kind="ExternalInput").ap()

    out_sl = nc.dram_tensor("out_sl", [S, OSH], f32, kind="ExternalOutput").ap()
    k_out = nc.dram_tensor("k_out", [128, S], f32, kind="ExternalOutput").ap()
    v_out = nc.dram_tensor("v_out", [128, S], f32, kind="ExternalOutput").ap()

    RG = [list(range(num_devices))]

    from contextlib import ExitStack

    with tile.TileContext(nc) as tc, ExitStack() as ctx:
        const = ctx.enter_context(tc.tile_pool(name="const", bufs=1))
        bigw = ctx.enter_context(tc.tile_pool(name="bigw", bufs=1))
        hsp = ctx.enter_context(tc.tile_pool(name="hsp", bufs=2))
        persist = ctx.enter_context(tc.tile_pool(name="persist", bufs=1))
        work = ctx.enter_context(tc.tile_pool(name="work", bufs=2))
        expp = ctx.enter_context(tc.tile_pool(name="expp", bufs=4))
        abp = ctx.enter_context(tc.tile_pool(name="abp", bufs=4))
        psum_acc = ctx.enter_context(
            tc.tile_pool(name="psum_acc", bufs=3, space="PSUM"))
        psum_sc = ctx.enter_context(
            tc.tile_pool(name="psum_sc", bufs=4, space="PSUM"))
        psum_sm = ctx.enter_context(
            tc.tile_pool(name="psum_sm", bufs=1, space="PSUM"))
        dram = ctx.enter_context(tc.tile_pool(name="dram", bufs=16, space="DRAM"))

        # ---- constants ----
        # DMA emission order = approximate issue order; load in the order
        # compute needs the data: wk/wv -> first hs block -> wq -> rest.
        wk_sb = const.tile([128, 32, HD], bf16)
        nc.sync.dma_start(wk_sb[:], wk[:])
        wv_sb = const.tile([128, 32, HD], bf16)
        nc.sync.dma_start(wv_sb[:], wv[:])

        hs0_sb = hsp.tile([128, 32, SB], bf16, tag="hs")
        for hc in range(8):
            nc.sync.dma_start(hs0_sb[:, ts(hc, 4), :],
                              hsT[:, ts(hc, 4), ts(0, SB)])

        # big weights share one slot: wq used in phase 1, dw in phase 3
        wq_sb = bigw.tile([128, 32, OSH], bf16, tag="bigw")
        nc.sync.dma_start(wq_sb[:], wq[:])

        cos_sb = const.tile([128, S], bf16)
        nc.sync.dma_start(cos_sb[:], cosT[:])
        sin_sb = const.tile([128, S], bf16)
        nc.sync.dma_start(sin_sb[:], sinT[:])
        mask_sb = const.tile([128, 4, SB], bf16)
        nc.sync.dma_start(mask_sb[:], maskM[:])
        ones_sb = const.tile([128, 128], bf16)
        nc.gpsimd.memset(ones_sb[:], 1.0)
        ident = const.tile([128, 128], bf16)
        make_identity(nc, ident[:])
        rot_sb = const.tile([128, 128], bf16)
        nc.sync.dma_start(rot_sb[:], rotm[:])

        # persistent activations
        qr_sb = persist.tile([128, HPC, S], bf16)  # RoPE'd qT per local head
        kr_sb = persist.tile([128, S], bf16)       # RoPE'd kT
        vt_sb = persist.tile([128, S], bf16)       # vT
        vn_sb = persist.tile([128, NKB, HD], bf16)  # v natural [sk, d] blocks

        def rope(dst, src, js):
            # dst/src [128, 512] bf16. rotate_half(x) done as a PE matmul
            # with the constant +/-1 rotation matrix (DVE lanes cannot read
            # across partitions), then two aligned muls + add.
            c = cos_sb[:, ts(js, SB)]
            si = sin_sb[:, ts(js, SB)]
            pr = psum_acc.tile([128, SB], f32, tag="acc")
            nc.tensor.matmul(pr[:], rot_sb[:], src[:], start=True, stop=True)
            rb = work.tile([128, SB], bf16, tag="rope_b")
            nc.any.tensor_copy(rb[:], pr[:])
            a = work.tile([128, SB], bf16, tag="rope_a")
            nc.vector.tensor_mul(a[:], src[:], c)
            nc.vector.tensor_mul(rb[:], rb[:], si)
            nc.vector.tensor_add(dst[:], a[:], rb[:])

        pending_rope = []

        # ---- phase 1: QKV projections + RoPE ----
        # All 6 projection groups for an s-block are emitted first; the RoPE
        # matmuls (which depend on cross-engine PSUM->SBUF copies) follow, so
        # the PE never head-of-line blocks on a copy.
        for j in range(NSB):
            if j == 0:
                hs_sb = hs0_sb
            else:
                hs_sb = hsp.tile([128, 32, SB], bf16, tag="hs")
                for hc in range(8):
                    nc.sync.dma_start(hs_sb[:, ts(hc, 4), :],
                                      hsT[:, ts(hc, 4), ts(j, SB)])

            # k projection
            pk = psum_acc.tile([128, SB], f32, tag="acc")
            for ho in range(32):
                nc.tensor.matmul(pk[:], wk_sb[:, ho, :], hs_sb[:, ho, :],
                                 start=(ho == 0), stop=(ho == 31))
            kraw = work.tile([128, SB], bf16, tag="kraw")
            nc.any.tensor_copy(kraw[:], pk[:])

            # v projection
            pv = psum_acc.tile([128, SB], f32, tag="acc")
            for ho in range(32):
                nc.tensor.matmul(pv[:], wv_sb[:, ho, :], hs_sb[:, ho, :],
                                 start=(ho == 0), stop=(ho == 31))
            v32 = work.tile([128, SB], f32, tag="k32")
            nc.any.tensor_copy(v32[:], pv[:])
            nc.sync.dma_start(v_out[:, ts(j, SB)], v32[:])
            nc.any.tensor_copy(vt_sb[:, ts(j, SB)], pv[:])

            # previous s-block's RoPE now; its copies are long done
            while pending_rope:
                pending_rope.pop(0)()

            # q projections (4 local heads)
            qraws = []
            for t in range(HPC):
                pq = psum_acc.tile([128, SB], f32, tag="acc")
                for ho in range(32):
                    nc.tensor.matmul(pq[:], wq_sb[:, ho, ts(t, HD)],
                                     hs_sb[:, ho, :],
                                     start=(ho == 0), stop=(ho == 31))
                qraw = work.tile([128, SB], bf16, tag="qraw", bufs=5,
                                 name=f"qraw_{j}_{t}")
                nc.any.tensor_copy(qraw[:], pq[:])
                qraws.append(qraw)

            # RoPE deferred into the next s-block's projection stream so
            # the rope matmuls never wait on the PSUM->SBUF copies.
            def make_ropes(j=j, kraw=kraw, qraws=qraws):
                rope(kr_sb[:, ts(j, SB)], kraw, j)
                k32 = work.tile([128, SB], f32, tag="k32",
                                name=f"k32_{j}")
                nc.any.tensor_copy(k32[:], kr_sb[:, ts(j, SB)])
                nc.sync.dma_start(k_out[:, ts(j, SB)], k32[:])
                for t in range(HPC):
                    rope(qr_sb[:, t, ts(j, SB)], qraws[t], j)
            pending_rope.append(make_ropes)

            # v transpose for this s-block (4 sk blocks of 128)
            for ii in range(4):
                i = 4 * j + ii
                pt = psum_acc.tile([128, HD], bf16, tag="acc",
                                   padded_shape=[128, SB])
                nc.tensor.transpose(pt[:], vt_sb[:, ts(i, HD)], ident[:])
                nc.any.tensor_copy(vn_sb[:, i, :], pt[:])

        # ---- phase 2: attention per head ----
        # AllGather bounce buffers, one per (local head, seq half)
        agin = [[dram.tile([128, 2 * SB], bf16, name=f"agin_{t}_{hf}")
                 for hf in range(2)] for t in range(HPC)]
        agout = [[dram.tile([NCORES * 128, 2 * SB], bf16,
                            addr_space="Shared", name=f"agout_{t}_{hf}")
                  for hf in range(2)] for t in range(HPC)]

        pending_fin = []

        def flush_fin():
            while pending_fin:
                pending_fin.pop(0)()

        def attention(t, j):
            nblk = 4 * j + 4  # causal sk blocks
            pat = psum_acc.tile([128, SB], f32, tag="acc",
                                name=f"pat_{t}_{j}")
            psm = psum_sm.tile([1, SB], f32, tag="sm", name=f"psm_{t}_{j}")
            ests = {}
            for i in range(nblk + 1):
                if i < nblk:
                    psc = psum_sc.tile([128, SB], f32, tag="sc",
                                       name=f"psc_{t}_{j}_{i}")
                    nc.tensor.matmul(psc[:], kr_sb[:, ts(i, HD)],
                                     qr_sb[:, t, ts(j, SB)],
                                     start=True, stop=True)
                    est = expp.tile([128, SB], bf16, tag="est",
                                    name=f"est_{t}_{j}_{i}")
                    nc.scalar.activation(est[:], psc[:],
                                         mybir.ActivationFunctionType.Exp,
                                         scale=SCALE)
                    if i >= 4 * j:  # diagonal block: causal 0/1 mask
                        nc.vector.tensor_mul(est[:], est[:],
                                             mask_sb[:, i - 4 * j, :])
                    ests[i] = est
                if i > 0:
                    # one block behind the scores: PE always has the next
                    # scores matmul queued while this est finishes on ACT.
                    e = ests.pop(i - 1)
                    nc.tensor.matmul(psm[:], ones_sb[:, 0:1], e[:],
                                     start=(i == 1), stop=(i == nblk))
                    nc.tensor.matmul(pat[:], vn_sb[:, i - 1, :], e[:],
                                     start=(i == 1), stop=(i == nblk))
                if i == 1:
                    # previous block's finalize: its reciprocal chain has had
                    # a full block of PE work to complete, so the broadcast
                    # matmul below won't head-of-line block the PE.
                    flush_fin()

            def finalize():
                rec = work.tile([1, SB], f32, tag="rec", name=f"rec_{t}_{j}")
                nc.vector.reciprocal(rec[:], psm[:])
                recb = work.tile([1, SB], bf16, tag="recb",
                                 name=f"recb_{t}_{j}")
                nc.any.tensor_copy(recb[:], rec[:])
                pbc = psum_acc.tile([128, SB], f32, tag="acc",
                                    name=f"pbc_{t}_{j}")
                nc.tensor.matmul(pbc[:], ones_sb[0:1, :], recb[:],
                                 start=True, stop=True)
                atf = work.tile([128, SB], f32, tag="atf",
                                name=f"atf_{t}_{j}")
                nc.any.tensor_copy(atf[:], pat[:])
                atb = work.tile([128, SB], bf16, tag="atb",
                                name=f"atb_{t}_{j}")
                nc.vector.tensor_mul(atb[:], atf[:], pbc[:])
                nc.sync.dma_start(agin[t][j // 2][:, ts(j % 2, SB)], atb[:])

            pending_fin.append(finalize)

        def allgather(t, hf):
            if num_devices == NCORES:
                nc.gpsimd.collective_compute(
                    "AllGather", mybir.AluOpType.bypass, replica_groups=RG,
                    ins=[agin[t][hf].opt()], outs=[agout[t][hf].opt()])
            else:
                # single-core sim variant: stand-in copy keeps the dense
                # dependencies representative.
                nc.sync.dma_start(agout[t][hf][0:128, :], agin[t][hf][:])

        def dense_chunk(hf, sqc):
            # one 128-row sq chunk of this half: out rows q16*128..
            q16 = hf * 8 + sqc
            pd = psum_acc.tile([128, OSH], f32, tag="acc",
                               name=f"pd_{hf}_{sqc}")
            first = True
            for t in range(HPC):
                agv = agout[t][hf].rearrange("(c p) s -> p c s", p=128)
                ab = abp.tile([128, NCORES, HD], bf16, tag="ab",
                              name=f"ab_{hf}_{sqc}_{t}")
                nc.sync.dma_start(ab[:], agv[:, :, ts(sqc, HD)])
                for c in range(NCORES):
                    gh = HPC * c + t
                    nc.tensor.matmul(pd[:], ab[:, c, :], dw_sb[:, gh, :],
                                     start=first,
                                     stop=(t == HPC - 1 and c == NCORES - 1))
                    first = False
            of = work.tile([128, OSH], f32, tag="of", name=f"of_{hf}_{sqc}")
            nc.any.tensor_copy(of[:], pd[:])
            nc.sync.dma_start(out_sl[ts(q16, 128), :], of[:])

        while pending_rope:
            pending_rope.pop(0)()

        # half 0 attention for all heads; AG per head once its finalizes
        # (deferred into the next attention block) have been emitted
        for t in range(HPC):
            attention(t, 0)
            if t > 0:
                allgather(t - 1, 0)
            attention(t, 1)
        flush_fin()
        allgather(HPC - 1, 0)

        # dense weights load (wq slot is free now; overlaps attention)
        dw_sb = bigw.tile([128, 32, OSH], bf16, tag="bigw")
        nc.sync.dma_start(dw_sb[:], dw[:])

        # half 1 attention with dense half-0 chunks interleaved
        for t in range(HPC):
            attention(t, 2)
            if t > 0:
                allgather(t - 1, 1)
                dense_chunk(0, 2 * (t - 1))
                dense_chunk(0, 2 * (t - 1) + 1)
            attention(t, 3)
        flush_fin()
        allgather(HPC - 1, 1)
        dense_chunk(0, 6)
        dense_chunk(0, 7)

        # dense half 1
        for sqc in range(8):
            dense_chunk(1, sqc)

    nc.compile()
    return nc


_BUILT = None


def _get_nc():
    global _BUILT
    if _BUILT is None:
        _BUILT = _build()
    return _BUILT


def _prep_inputs(hidden_states, cos, sin, wq_w, wk_w, wv_w, dense_w):
    """Host-side shard prep: transposes + bf16 casts + per-core slices."""
    hs = np.asarray(hidden_states, np.float32).reshape(S, H)
    hsT = np.ascontiguousarray(hs.T).reshape(32, 128, S).transpose(1, 0, 2)
    hsT = np.ascontiguousarray(hsT).astype(BF16)

    def wslice(wT, lo, hi):
        # wT [H, m] -> [128, 32, m] partition-major
        w = np.ascontiguousarray(wT[:, lo:hi]).reshape(32, 128, hi - lo)
        return np.ascontiguousarray(w.transpose(1, 0, 2)).astype(BF16)

    wqT = np.asarray(wq_w, np.float32).T  # [H, H]
    wkT = np.asarray(wk_w, np.float32).T  # [H, HD]
    wvT = np.asarray(wv_w, np.float32).T
    dwT = np.asarray(dense_w, np.float32).T  # [H(hd), H(o)]

    wk_l = wslice(wkT, 0, HD)
    wv_l = wslice(wvT, 0, HD)
    cosT = np.ascontiguousarray(
        np.asarray(cos, np.float32).reshape(S, HD).T).astype(BF16)
    sinT = np.ascontiguousarray(
        np.asarray(sin, np.float32).reshape(S, HD).T).astype(BF16)
    # mask tiles: M[r',c] = 1 if r' <= c (allowed), [512,512] -> [128,4,512]
    r = np.arange(SB)
    M = (r[:, None] <= r[None, :]).astype(BF16)
    maskM = np.ascontiguousarray(
        M.reshape(4, 128, SB).transpose(1, 0, 2))
    # rotation matrix: (R @ q)[d] = -q[d+64] (d<64), q[d-64] (d>=64);
    # device matmul computes lhsT.T @ rhs, so feed R.T.
    R = np.zeros((128, 128), np.float32)
    for d in range(64):
        R[d, 64 + d] = -1.0
        R[64 + d, d] = 1.0
    rotm = np.ascontiguousarray(R.T).astype(BF16)

    in_maps = []
    for c in range(NCORES):
        in_maps.append({
            "hsT": hsT,
            "wq": wslice(wqT, OSH * c, OSH * (c + 1)),
            "wk": wk_l,
            "wv": wv_l,
            "dw": wslice(dwT, OSH * c, OSH * (c + 1)),
            "cosT": cosT,
            "sinT": sinT,
            "maskM": maskM,
            "rotm": rotm,
        })
    return in_maps


_RUNNER = None


def _get_runner():
    """Build (once) a jitted shard_map callable over the 8 cores, mirroring
    concourse.bass2jax.run_bass_via_pjrt's multi-core branch, so repeated
    calls reuse the compiled NEFF and we can time executions."""
    global _RUNNER
    if _RUNNER is not None:
        return _RUNNER
    import jax
    import concourse.mybir as mybir
    from jax.sharding import Mesh, PartitionSpec
    from jax.experimental.shard_map import shard_map
    from concourse.bass2jax import (_bass_exec_p, install_neuronx_cc_hook,
                                    partition_id_tensor)

    nc = _get_nc()
    install_neuronx_cc_hook()
    partition_name = (nc.partition_id_tensor.name
                      if nc.partition_id_tensor else None)

    in_names, out_names, out_avals, zero_outs = [], [], [], []
    for alloc in nc.m.functions[0].allocations:
        if not isinstance(alloc, mybir.MemoryLocationSet):
            continue
        name = alloc.memorylocations[0].name
        if alloc.kind == "ExternalInput":
            if name != partition_name:
                in_names.append(name)
        elif alloc.kind == "ExternalOutput":
            out_names.append(name)
            shape = tuple(alloc.tensor_shape)
            dtype = mybir.dt.np(alloc.dtype)
            out_avals.append(jax.core.ShapedArray(shape, dtype))
            zero_outs.append(np.zeros(shape, dtype))
    n_params = len(in_names)
    n_outs = len(out_avals)
    all_in_names = list(in_names) + list(out_names)
    if partition_name is not None:
        all_in_names.append(partition_name)

    def _body(*args):
        operands = list(args)
        if partition_name is not None:
            operands.append(partition_id_tensor())
        outs = _bass_exec_p.bind(
            *operands,
            out_avals=tuple(out_avals),
            in_names=tuple(all_in_names),
            out_names=tuple(out_names),
            lowering_input_output_aliases=(),
            sim_require_finite=True,
            sim_require_nnan=True,
            nc=nc,
        )
        return tuple(outs)

    devices = jax.devices()[:NCORES]
    mesh = Mesh(np.asarray(devices), ("core",))
    in_specs = (PartitionSpec("core"),) * (n_params + n_outs)
    out_specs = (PartitionSpec("core"),) * n_outs
    donate = tuple(range(n_params, n_params + n_outs))
    fn = jax.jit(
        shard_map(_body, mesh=mesh, in_specs=in_specs, out_specs=out_specs,
                  check_rep=False),
        donate_argnums=donate, keep_unused=True)
    _RUNNER = dict(fn=fn, mesh=mesh, in_names=in_names, out_names=out_names,
                   out_avals=out_avals, zero_outs=zero_outs,
                   n_params=n_params)
    return _RUNNER


def _concat_inputs(in_maps):
    r = _get_runner()
    return [np.concatenate([np.asarray(in_maps[c][name])
                            for c in range(NCORES)], axis=0)
            for name in r["in_names"]]


def _zero_bufs():
    r = _get_runner()
    return [np.zeros((NCORES * z.shape[0], *z.shape[1:]), z.dtype)
            for z in r["zero_outs"]]


def _run(in_maps):
    r = _get_runner()
    out_arrs = r["fn"](*_concat_inputs(in_maps), *_zero_bufs())
    res = []
    for c in range(NCORES):
        res.append({name: np.asarray(out_arrs[i]).reshape(
            NCORES, *r["out_avals"][i].shape)[c]
            for i, name in enumerate(r["out_names"])})
    return res


def kernel(hidden_states, cos, sin, wq_w, wk_w, wv_w, dense_w):
    in_maps = _prep_inputs(hidden_states, cos, sin, wq_w, wk_w, wv_w, dense_w)
    res = _run(in_maps)
    out = np.concatenate([res[c]["out_sl"] for c in range(NCORES)], axis=1)
    out = out.reshape(1, S, H)
    key = res[0]["k_out"].T.reshape(1, 1, S, HD)
    value = res[0]["v_out"].T.reshape(1, 1, S, HD)
    return (out, key, value)


def _make_chained_fn(K):
    """jit fn that executes the NEFF K times back-to-back (separate zero
    buffers per execution) so per-dispatch axon overhead amortizes."""
    import jax
    from jax.sharding import PartitionSpec
    from jax.experimental.shard_map import shard_map
    from concourse.bass2jax import _bass_exec_p, partition_id_tensor

    r = _get_runner()
    nc = _get_nc()
    partition_name = (nc.partition_id_tensor.name
                      if nc.partition_id_tensor else None)
    out_names = r["out_names"]
    out_avals = r["out_avals"]
    in_names = r["in_names"]
    n_params = r["n_params"]
    n_outs = len(out_names)
    all_in_names = list(in_names) + list(out_names)
    if partition_name is not None:
        all_in_names.append(partition_name)

    def _body(*args):
        ins = list(args[:n_params])
        outs_all = []
        for k in range(K):
            zeros_k = list(args[n_params + k * n_outs:
                                n_params + (k + 1) * n_outs])
            operands = ins + zeros_k
            if partition_name is not None:
                operands.append(partition_id_tensor())
            outs = _bass_exec_p.bind(
                *operands,
                out_avals=tuple(out_avals),
                in_names=tuple(all_in_names),
                out_names=tuple(out_names),
                lowering_input_output_aliases=(),
                sim_require_finite=True,
                sim_require_nnan=True,
                nc=nc,
            )
            outs_all.extend(outs)
        return tuple(outs_all)

    mesh = r["mesh"]
    nin = n_params + K * n_outs
    fn = jax.jit(
        shard_map(_body, mesh=mesh,
                  in_specs=(PartitionSpec("core"),) * nin,
                  out_specs=(PartitionSpec("core"),) * (K * n_outs),
                  check_rep=False),
        donate_argnums=tuple(range(n_params, nin)), keep_unused=True)
    return fn


# revision 24
# speedup vs baseline: 1.6991x; 1.0766x over previous
"""MQA attention kernel for 8 Trainium2 NeuronCores.

Sharding: tensor-parallel over the 32 query heads (4 heads per core), shared
K/V head replicated. After per-head attention, the per-head outputs
(attnT [head_dim, S]) are AllGathered; the dense projection is column-sharded
(each core computes 512 of the 4096 output features for all tokens), so the
host-side unshard is a concat, with no cross-core reduction.

All matmuls run in bf16 with fp32 PSUM accumulation.

Device layouts (partition dim first):
  hsT   [128 h_in, 32 h_out, 2048 s]   transposed hidden states (replicated)
  wq    [128 h_in, 32 h_out, 512  m]   wq_w.T slice for this core's 4 heads
  wk/wv [128 h_in, 32 h_out, 128  d]   shared K/V projections (replicated)
  dw    [128 hd_in, 32 head, 512  o]   dense_w.T column slice (this core's o)
  cosT/sinT [128 d, 2048 s]
  mask  [128 r, 4 delta, 512 c]        causal 0/1 tiles for diagonal blocks

Per-core compute:
  qT/kT projections -> RoPE -> scoresT [sk,sq] = kT.T @ qT -> exp (scale folded
  into ACT) -> causal mask multiply -> column sums via ones-matmul ->
  attnT [d,sq] = v.T-free accumulate -> normalize -> AllGather -> dense.
"""

import numpy as np
import ml_dtypes

S = 2048
H = 4096
NH = 32
HD = 128
NCORES = 8
HPC = NH // NCORES  # 4 heads per core
OSH = H // NCORES  # 512 output features per core
SB = 512  # sequence block for matmul free dim
SKV = S // NCORES  # 256: per-core K/V sequence slice
NSB = S // SB  # 4
NKB = S // HD  # 16 sk blocks of 128
SCALE = 1.0 / float(HD) ** 0.5

BF16 = ml_dtypes.bfloat16


def _build(num_devices=NCORES):
    import concourse.bass as bass
    import concourse.tile as tile
    import concourse.mybir as mybir
    from concourse import bacc
    from concourse.bass import ts
    from concourse.masks import make_identity

    f32 = mybir.dt.float32
    bf16 = mybir.dt.bfloat16

    nc = bacc.Bacc("TRN2", target_bir_lowering=False, debug=False,
                   num_devices=num_devices)

    hsT = nc.dram_tensor("hsT", [128, 32, S], bf16, kind="ExternalInput").ap()
    wq = nc.dram_tensor("wq", [128, 32, OSH], bf16, kind="ExternalInput").ap()
    wk = nc.dram_tensor("wk", [128, 32, HD], bf16, kind="ExternalInput").ap()
    wv = nc.dram_tensor("wv", [128, 32, HD], bf16, kind="ExternalInput").ap()
    dw = nc.dram_tensor("dw", [128, 32, OSH], bf16, kind="ExternalInput").ap()
    cosT = nc.dram_tensor("cosT", [128, S], bf16, kind="ExternalInput").ap()
    sinT = nc.dram_tensor("sinT", [128, S], bf16, kind="ExternalInput").ap()
    maskM = nc.dram_tensor("maskM", [128, 4, SB], bf16, kind="ExternalInput").ap()
    rotm = nc.dram_tensor("rotm", [128, 128], bf16, kind="ExternalInput").ap()
    hs_kv = nc.dram_tensor("hs_kv", [128, 32, SKV], bf16,
                           kind="ExternalInput").ap()
    cos_kv = nc.dram_tensor("cos_kv", [128, SKV], bf16,
                            kind="ExternalInput").ap()
    sin_kv = nc.dram_tensor("sin_kv", [128, SKV], bf16,
                            kind="ExternalInput").ap()

    out_sl = nc.dram_tensor("out_sl", [S, OSH], f32, kind="ExternalOutput").ap()
    k_out = nc.dram_tensor("k_out", [128, S], f32, kind="ExternalOutput").ap()
    v_out = nc.dram_tensor("v_out", [S, HD], f32, kind="ExternalOutput").ap()

    RG = [list(range(num_devices))]

    from contextlib import ExitStack

    with tile.TileContext(nc) as tc, ExitStack() as ctx:
        const = ctx.enter_context(tc.tile_pool(name="const", bufs=1))
        bigw = ctx.enter_context(tc.tile_pool(name="bigw", bufs=1))
        hsp = ctx.enter_context(tc.tile_pool(name="hsp", bufs=2))
        persist = ctx.enter_context(tc.tile_pool(name="persist", bufs=1))
        work = ctx.enter_context(tc.tile_pool(name="work", bufs=2))
        expp = ctx.enter_context(tc.tile_pool(name="expp", bufs=4))
        abp = ctx.enter_context(tc.tile_pool(name="abp", bufs=4))
        psum_acc = ctx.enter_context(
            tc.tile_pool(name="psum_acc", bufs=3, space="PSUM"))
        psum_sc = ctx.enter_context(
            tc.tile_pool(name="psum_sc", bufs=4, space="PSUM"))
        psum_sm = ctx.enter_context(
            tc.tile_pool(name="psum_sm", bufs=1, space="PSUM"))
        dram = ctx.enter_context(tc.tile_pool(name="dram", bufs=16, space="DRAM"))

        # ---- constants ----
        # DMA emission order = approximate issue order; load in the order
        # compute needs the data: wk/wv -> first hs block -> wq -> rest.
        wk_sb = const.tile([128, 32, HD], bf16)
        nc.sync.dma_start(wk_sb[:], wk[:])
        wv_sb = const.tile([128, 32, HD], bf16)
        nc.sync.dma_start(wv_sb[:], wv[:])
        coskv_sb = const.tile([128, SKV], bf16)
        nc.sync.dma_start(coskv_sb[:], cos_kv[:])
        sinkv_sb = const.tile([128, SKV], bf16)
        nc.sync.dma_start(sinkv_sb[:], sin_kv[:])
        rot_sb = const.tile([128, 128], bf16)
        nc.sync.dma_start(rot_sb[:], rotm[:])
        ident = const.tile([128, 128], bf16)
        make_identity(nc, ident[:])

        hs0_sb = hsp.tile([128, 32, SB], bf16, tag="hs")
        for hc in range(8):
            nc.sync.dma_start(hs0_sb[:, ts(hc, 4), :],
                              hsT[:, ts(hc, 4), ts(0, SB)])

        # big weights share one slot: wq used in phase 1, dw in phase 3
        wq_sb = bigw.tile([128, 32, OSH], bf16, tag="bigw")
        nc.sync.dma_start(wq_sb[:], wq[:])

        cos_sb = const.tile([128, S], bf16)
        nc.sync.dma_start(cos_sb[:], cosT[:])
        sin_sb = const.tile([128, S], bf16)
        nc.sync.dma_start(sin_sb[:], sinT[:])
        mask_sb = const.tile([128, 4, SB], bf16)
        nc.sync.dma_start(mask_sb[:], maskM[:])
        ones_sb = const.tile([128, 128], bf16)
        nc.gpsimd.memset(ones_sb[:], 1.0)

        # persistent activations
        qr_sb = persist.tile([128, HPC, S], bf16)  # RoPE'd qT per local head
        kr_sb = persist.tile([128, NCORES, SKV], bf16)   # RoPE'd kT (from AG)
        vn_sb = persist.tile([128, NCORES, 2, HD], bf16)  # v natural (from AG)

        def rope(dst, src, js):
            # dst/src [128, 512] bf16. rotate_half(x) done as a PE matmul
            # with the constant +/-1 rotation matrix (DVE lanes cannot read
            # across partitions), then two aligned muls + add.
            c = cos_sb[:, ts(js, SB)]
            si = sin_sb[:, ts(js, SB)]
            pr = psum_acc.tile([128, SB], f32, tag="acc")
            nc.tensor.matmul(pr[:], rot_sb[:], src[:], start=True, stop=True)
            rb = work.tile([128, SB], bf16, tag="rope_b")
            nc.any.tensor_copy(rb[:], pr[:])
            a = work.tile([128, SB], bf16, tag="rope_a")
            nc.vector.tensor_mul(a[:], src[:], c)
            nc.vector.tensor_mul(rb[:], rb[:], si)
            nc.vector.tensor_add(dst[:], a[:], rb[:])

        pending_rope = []

        # ---- phase 0.5: K/V for this core's 256-token slice + AllGather ----
        # Each core projects K/V only for its S/8 slice (hs_kv input), RoPEs
        # K, transposes V to natural layout, and two small AllGathers
        # distribute the full K/V while the q projections run.
        agin_k = dram.tile([128, SKV], bf16, name="agin_k")
        agout_k = dram.tile([NCORES * 128, SKV], bf16, addr_space="Shared",
                            name="agout_k")
        agin_v = dram.tile([128, SKV], bf16, name="agin_v")
        agout_v = dram.tile([NCORES * 128, SKV], bf16, addr_space="Shared",
                            name="agout_v")

        pkk = psum_acc.tile([128, SKV], f32, tag="acc", padded_shape=[128, SB])
        pvv = psum_acc.tile([128, SKV], f32, tag="acc", padded_shape=[128, SB])
        for hop in range(4):
            hkv = hsp.tile([128, 8, SKV], bf16, tag="hkv", bufs=2,
                           name=f"hkv_{hop}")
            nc.sync.dma_start(hkv[:], hs_kv[:, ts(hop, 8), :])
            for ho8 in range(8):
                ho = hop * 8 + ho8
                nc.tensor.matmul(pkk[:], wk_sb[:, ho, :], hkv[:, ho8, :],
                                 start=(ho == 0), stop=(ho == 31),
                                 skip_group_check=True)
            for ho8 in range(8):
                ho = hop * 8 + ho8
                nc.tensor.matmul(pvv[:], wv_sb[:, ho, :], hkv[:, ho8, :],
                                 start=(ho == 0), stop=(ho == 31),
                                 skip_group_check=True)
        # RoPE the k slice (uses the per-core cos/sin slice)
        kraw_s = work.tile([128, SKV], bf16, tag="kraw")
        nc.any.tensor_copy(kraw_s[:], pkk[:])
        prk = psum_acc.tile([128, SKV], f32, tag="acc", padded_shape=[128, SB])
        nc.tensor.matmul(prk[:], rot_sb[:], kraw_s[:], start=True, stop=True)
        rbk = work.tile([128, SKV], bf16, tag="rope_b")
        nc.any.tensor_copy(rbk[:], prk[:])
        ak = work.tile([128, SKV], bf16, tag="rope_a")
        nc.vector.tensor_mul(ak[:], kraw_s[:], coskv_sb[:])
        nc.vector.tensor_mul(rbk[:], rbk[:], sinkv_sb[:])
        krs = work.tile([128, SKV], bf16, tag="krs")
        nc.vector.tensor_add(krs[:], ak[:], rbk[:])
        nc.sync.dma_start(agin_k[:], krs[:])

        # V slice -> natural layout (transpose the two 128-blocks)
        vts = work.tile([128, SKV], bf16, tag="krs")
        nc.any.tensor_copy(vts[:], pvv[:])
        vns = work.tile([128, 2, HD], bf16, tag="vns")
        for b in range(2):
            ptv = psum_acc.tile([128, HD], bf16, tag="acc",
                                padded_shape=[128, SB], name=f"ptv_{b}")
            nc.tensor.transpose(ptv[:], vts[:, ts(b, HD)], ident[:])
            nc.any.tensor_copy(vns[:, b, :], ptv[:])
        nc.sync.dma_start(agin_v[:], vns[:].rearrange("p b d -> p (b d)"))

        if num_devices == NCORES:
            nc.gpsimd.collective_compute(
                "AllGather", mybir.AluOpType.bypass, replica_groups=RG,
                ins=[agin_k.opt()], outs=[agout_k.opt()])
            nc.gpsimd.collective_compute(
                "AllGather", mybir.AluOpType.bypass, replica_groups=RG,
                ins=[agin_v.opt()], outs=[agout_v.opt()])
        else:
            nc.sync.dma_start(agout_k[0:128, :], agin_k[:])
            nc.sync.dma_start(agout_v[0:128, :], agin_v[:])

        # gathered K/V into SBUF (waits on the AG semaphores when they run)
        nc.sync.dma_start(kr_sb[:],
                          agout_k.rearrange("(c p) s -> p c s", p=128))
        nc.sync.dma_start(vn_sb[:],
                          agout_v.rearrange("(c p) (b d) -> p c b d",
                                            p=128, b=2))

        # ---- phase 1: QKV projections + RoPE ----
        # All 6 projection groups for an s-block are emitted first; the RoPE
        # matmuls (which depend on cross-engine PSUM->SBUF copies) follow, so
        # the PE never head-of-line blocks on a copy.
        for j in range(NSB):
            if j == 0:
                hs_sb = hs0_sb
            else:
                hs_sb = hsp.tile([128, 32, SB], bf16, tag="hs")
                for hc in range(8):
                    nc.sync.dma_start(hs_sb[:, ts(hc, 4), :],
                                      hsT[:, ts(hc, 4), ts(j, SB)])

            # previous s-block's RoPE now; its copies are long done
            while pending_rope:
                pending_rope.pop(0)()

            # q projections (4 local heads)
            qraws = []
            for t in range(HPC):
                pq = psum_acc.tile([128, SB], f32, tag="acc")
                for ho in range(32):
                    nc.tensor.matmul(pq[:], wq_sb[:, ho, ts(t, HD)],
                                     hs_sb[:, ho, :],
                                     start=(ho == 0), stop=(ho == 31))
                qraw = work.tile([128, SB], bf16, tag="qraw", bufs=5,
                                 name=f"qraw_{j}_{t}")
                nc.any.tensor_copy(qraw[:], pq[:])
                qraws.append(qraw)

            # RoPE deferred into the next s-block's projection stream so
            # the rope matmuls never wait on the PSUM->SBUF copies.
            def make_ropes(j=j, qraws=qraws):
                for t in range(HPC):
                    rope(qr_sb[:, t, ts(j, SB)], qraws[t], j)
            pending_rope.append(make_ropes)

        # ---- phase 2: attention per head ----
        # AllGather bounce buffers, one per (local head, seq half)
        agin = [[dram.tile([128, 2 * SB], bf16, name=f"agin_{t}_{hf}")
                 for hf in range(2)] for t in range(HPC)]
        agout = [[dram.tile([NCORES * 128, 2 * SB], bf16,
                            addr_space="Shared", name=f"agout_{t}_{hf}")
                  for hf in range(2)] for t in range(HPC)]

        pending_fin = []

        def flush_fin():
            while pending_fin:
                pending_fin.pop(0)()

        def attention(t, j):
            nblk = 4 * j + 4  # causal sk blocks
            pat = psum_acc.tile([128, SB], f32, tag="acc",
                                name=f"pat_{t}_{j}")
            psm = psum_sm.tile([1, SB], f32, tag="sm", name=f"psm_{t}_{j}")
            ests = {}
            for i in range(nblk + 1):
                if i < nblk:
                    psc = psum_sc.tile([128, SB], f32, tag="sc",
                                       name=f"psc_{t}_{j}_{i}")
                    nc.tensor.matmul(psc[:], kr_sb[:, i // 2, ts(i % 2, HD)],
                                     qr_sb[:, t, ts(j, SB)],
                                     start=True, stop=True)
                    est = expp.tile([128, SB], bf16, tag="est",
                                    name=f"est_{t}_{j}_{i}")
                    nc.scalar.activation(est[:], psc[:],
                                         mybir.ActivationFunctionType.Exp,
                                         scale=SCALE)
                    if i >= 4 * j:  # diagonal block: causal 0/1 mask
                        nc.vector.tensor_mul(est[:], est[:],
                                             mask_sb[:, i - 4 * j, :])
                    ests[i] = est
                if i > 0:
                    # one block behind the scores: PE always has the next
                    # scores matmul queued while this est finishes on ACT.
                    e = ests.pop(i - 1)
                    nc.tensor.matmul(psm[:], ones_sb[:, 0:1], e[:],
                                     start=(i == 1), stop=(i == nblk))
                    nc.tensor.matmul(pat[:],
                                     vn_sb[:, (i - 1) // 2, (i - 1) % 2, :],
                                     e[:],
                                     start=(i == 1), stop=(i == nblk))
                if i == 1:
                    # previous block's finalize: its reciprocal chain has had
                    # a full block of PE work to complete, so the broadcast
                    # matmul below won't head-of-line block the PE.
                    flush_fin()

            def finalize():
                rec = work.tile([1, SB], f32, tag="rec", name=f"rec_{t}_{j}")
                nc.vector.reciprocal(rec[:], psm[:])
                recb = work.tile([1, SB], bf16, tag="recb",
                                 name=f"recb_{t}_{j}")
                nc.any.tensor_copy(recb[:], rec[:])
                pbc = psum_acc.tile([128, SB], f32, tag="acc",
                                    name=f"pbc_{t}_{j}")
                nc.tensor.matmul(pbc[:], ones_sb[0:1, :], recb[:],
                                 start=True, stop=True)
                atf = work.tile([128, SB], f32, tag="atf",
                                name=f"atf_{t}_{j}")
                nc.any.tensor_copy(atf[:], pat[:])
                atb = work.tile([128, SB], bf16, tag="atb",
                                name=f"atb_{t}_{j}")
                nc.vector.tensor_mul(atb[:], atf[:], pbc[:])
                nc.sync.dma_start(agin[t][j // 2][:, ts(j % 2, SB)], atb[:])

            pending_fin.append(finalize)

        def allgather(t, hf):
            if num_devices == NCORES:
                nc.gpsimd.collective_compute(
                    "AllGather", mybir.AluOpType.bypass, replica_groups=RG,
                    ins=[agin[t][hf].opt()], outs=[agout[t][hf].opt()])
            else:
                # single-core sim variant: stand-in copy keeps the dense
                # dependencies representative.
                nc.sync.dma_start(agout[t][hf][0:128, :], agin[t][hf][:])

        def dense_chunk(hf, sqc):
            # one 128-row sq chunk of this half: out rows q16*128..
            q16 = hf * 8 + sqc
            pd = psum_acc.tile([128, OSH], f32, tag="acc",
                               name=f"pd_{hf}_{sqc}")
            first = True
            for t in range(HPC):
                agv = agout[t][hf].rearrange("(c p) s -> p c s", p=128)
                ab = abp.tile([128, NCORES, HD], bf16, tag="ab",
                              name=f"ab_{hf}_{sqc}_{t}")
                nc.sync.dma_start(ab[:], agv[:, :, ts(sqc, HD)])
                for c in range(NCORES):
                    gh = HPC * c + t
                    nc.tensor.matmul(pd[:], ab[:, c, :], dw_sb[:, gh, :],
                                     start=first,
                                     stop=(t == HPC - 1 and c == NCORES - 1))
                    first = False
            of = work.tile([128, OSH], f32, tag="of", name=f"of_{hf}_{sqc}")
            nc.any.tensor_copy(of[:], pd[:])
            nc.sync.dma_start(out_sl[ts(q16, 128), :], of[:])

        while pending_rope:
            pending_rope.pop(0)()

        # K/V external outputs (f32 casts of the gathered buffers), in
        # chunks to bound SBUF. Low priority: emitted before attention but
        # they only occupy DVE/ACT + DMA.
        for cc in range(NCORES):
            ko = work.tile([128, SKV], f32, tag="ko", name=f"ko_{cc}")
            nc.any.tensor_copy(ko[:], kr_sb[:, cc, :])
            nc.sync.dma_start(k_out[:, cc * SKV:(cc + 1) * SKV], ko[:])
            vo = work.tile([128, 2, HD], f32, tag="vo", name=f"vo_{cc}")
            nc.any.tensor_copy(vo[:], vn_sb[:, cc, :, :])
            nc.sync.dma_start(
                v_out.rearrange("(c b p) d -> p c b d", p=128, b=2)
                [:, cc, :, :], vo[:])

        # half 0 attention for all heads; AG per head once its finalizes
        # (deferred into the next attention block) have been emitted
        for t in range(HPC):
            attention(t, 0)
            if t > 0:
                allgather(t - 1, 0)
            attention(t, 1)
        flush_fin()
        allgather(HPC - 1, 0)

        # dense weights load (wq slot is free now; overlaps attention)
        dw_sb = bigw.tile([128, 32, OSH], bf16, tag="bigw")
        nc.sync.dma_start(dw_sb[:], dw[:])

        # half 1 attention with dense half-0 chunks interleaved
        for t in range(HPC):
            attention(t, 2)
            if t > 0:
                allgather(t - 1, 1)
                dense_chunk(0, 2 * (t - 1))
                dense_chunk(0, 2 * (t - 1) + 1)
            attention(t, 3)
        flush_fin()
        allgather(HPC - 1, 1)
        dense_chunk(0, 6)
        dense_chunk(0, 7)

        # dense half 1
        for sqc in range(8):
            dense_chunk(1, sqc)

    nc.compile()
    return nc


_BUILT = None


def _get_nc():
    global _BUILT
    if _BUILT is None:
        _BUILT = _build()
    return _BUILT


def _prep_inputs(hidden_states, cos, sin, wq_w, wk_w, wv_w, dense_w):
    """Host-side shard prep: transposes + bf16 casts + per-core slices."""
    hs = np.asarray(hidden_states, np.float32).reshape(S, H)
    hsT = np.ascontiguousarray(hs.T).reshape(32, 128, S).transpose(1, 0, 2)
    hsT = np.ascontiguousarray(hsT).astype(BF16)

    def wslice(wT, lo, hi):
        # wT [H, m] -> [128, 32, m] partition-major
        w = np.ascontiguousarray(wT[:, lo:hi]).reshape(32, 128, hi - lo)
        return np.ascontiguousarray(w.transpose(1, 0, 2)).astype(BF16)

    wqT = np.asarray(wq_w, np.float32).T  # [H, H]
    wkT = np.asarray(wk_w, np.float32).T  # [H, HD]
    wvT = np.asarray(wv_w, np.float32).T
    dwT = np.asarray(dense_w, np.float32).T  # [H(hd), H(o)]

    wk_l = wslice(wkT, 0, HD)
    wv_l = wslice(wvT, 0, HD)
    cosT = np.ascontiguousarray(
        np.asarray(cos, np.float32).reshape(S, HD).T).astype(BF16)
    sinT = np.ascontiguousarray(
        np.asarray(sin, np.float32).reshape(S, HD).T).astype(BF16)
    # mask tiles: M[r',c] = 1 if r' <= c (allowed), [512,512] -> [128,4,512]
    r = np.arange(SB)
    M = (r[:, None] <= r[None, :]).astype(BF16)
    maskM = np.ascontiguousarray(
        M.reshape(4, 128, SB).transpose(1, 0, 2))
    # rotation matrix: (R @ q)[d] = -q[d+64] (d<64), q[d-64] (d>=64);
    # device matmul computes lhsT.T @ rhs, so feed R.T.
    R = np.zeros((128, 128), np.float32)
    for d in range(64):
        R[d, 64 + d] = -1.0
        R[64 + d, d] = 1.0
    rotm = np.ascontiguousarray(R.T).astype(BF16)

    in_maps = []
    for c in range(NCORES):
        sl = slice(SKV * c, SKV * (c + 1))
        in_maps.append({
            "hsT": hsT,
            "wq": wslice(wqT, OSH * c, OSH * (c + 1)),
            "wk": wk_l,
            "wv": wv_l,
            "dw": wslice(dwT, OSH * c, OSH * (c + 1)),
            "cosT": cosT,
            "sinT": sinT,
            "maskM": maskM,
            "rotm": rotm,
            "hs_kv": np.ascontiguousarray(hsT[:, :, sl]),
            "cos_kv": np.ascontiguousarray(cosT[:, sl]),
            "sin_kv": np.ascontiguousarray(sinT[:, sl]),
        })
    return in_maps


_RUNNER = None


def _get_runner():
    """Build (once) a jitted shard_map callable over the 8 cores, mirroring
    concourse.bass2jax.run_bass_via_pjrt's multi-core branch, so repeated
    calls reuse the compiled NEFF and we can time executions."""
    global _RUNNER
    if _RUNNER is not None:
        return _RUNNER
    import jax
    import concourse.mybir as mybir
    from jax.sharding import Mesh, PartitionSpec
    from jax.experimental.shard_map import shard_map
    from concourse.bass2jax import (_bass_exec_p, install_neuronx_cc_hook,
                                    partition_id_tensor)

    nc = _get_nc()
    install_neuronx_cc_hook()
    partition_name = (nc.partition_id_tensor.name
                      if nc.partition_id_tensor else None)

    in_names, out_names, out_avals, zero_outs = [], [], [], []
    for alloc in nc.m.functions[0].allocations:
        if not isinstance(alloc, mybir.MemoryLocationSet):
            continue
        name = alloc.memorylocations[0].name
        if alloc.kind == "ExternalInput":
            if name != partition_name:
                in_names.append(name)
        elif alloc.kind == "ExternalOutput":
            out_names.append(name)
            shape = tuple(alloc.tensor_shape)
            dtype = mybir.dt.np(alloc.dtype)
            out_avals.append(jax.core.ShapedArray(shape, dtype))
            zero_outs.append(np.zeros(shape, dtype))
    n_params = len(in_names)
    n_outs = len(out_avals)
    all_in_names = list(in_names) + list(out_names)
    if partition_name is not None:
        all_in_names.append(partition_name)

    def _body(*args):
        operands = list(args)
        if partition_name is not None:
            operands.append(partition_id_tensor())
        outs = _bass_exec_p.bind(
            *operands,
            out_avals=tuple(out_avals),
            in_names=tuple(all_in_names),
            out_names=tuple(out_names),
            lowering_input_output_aliases=(),
            sim_require_finite=True,
            sim_require_nnan=True,
            nc=nc,
        )
        return tuple(outs)

    devices = jax.devices()[:NCORES]
    mesh = Mesh(np.asarray(devices), ("core",))
    in_specs = (PartitionSpec("core"),) * (n_params + n_outs)
    out_specs = (PartitionSpec("core"),) * n_outs
    donate = tuple(range(n_params, n_params + n_outs))
    fn = jax.jit(
        shard_map(_body, mesh=mesh, in_specs=in_specs, out_specs=out_specs,
                  check_rep=False),
        donate_argnums=donate, keep_unused=True)
    _RUNNER = dict(fn=fn, mesh=mesh, in_names=in_names, out_names=out_names,
                   out_avals=out_avals, zero_outs=zero_outs,
                   n_params=n_params)
    return _RUNNER


def _concat_inputs(in_maps):
    r = _get_runner()
    return [np.concatenate([np.asarray(in_maps[c][name])
                            for c in range(NCORES)], axis=0)
            for name in r["in_names"]]


def _zero_bufs():
    r = _get_runner()
    return [np.zeros((NCORES * z.shape[0], *z.shape[1:]), z.dtype)
            for z in r["zero_outs"]]


def _run(in_maps):
    r = _get_runner()
    out_arrs = r["fn"](*_concat_inputs(in_maps), *_zero_bufs())
    res = []
    for c in range(NCORES):
        res.append({name: np.asarray(out_arrs[i]).reshape(
            NCORES, *r["out_avals"][i].shape)[c]
            for i, name in enumerate(r["out_names"])})
    return res


def kernel(hidden_states, cos, sin, wq_w, wk_w, wv_w, dense_w):
    in_maps = _prep_inputs(hidden_states, cos, sin, wq_w, wk_w, wv_w, dense_w)
    res = _run(in_maps)
    out = np.concatenate([res[c]["out_sl"] for c in range(NCORES)], axis=1)
    out = out.reshape(1, S, H)
    key = res[0]["k_out"].T.reshape(1, 1, S, HD)
    value = res[0]["v_out"].reshape(1, 1, S, HD)
    return (out, key, value)


def _make_chained_fn(K):
    """jit fn that executes the NEFF K times back-to-back (separate zero
    buffers per execution) so per-dispatch axon overhead amortizes."""
    import jax
    from jax.sharding import PartitionSpec
    from jax.experimental.shard_map import shard_map
    from concourse.bass2jax import _bass_exec_p, partition_id_tensor

    r = _get_runner()
    nc = _get_nc()
    partition_name = (nc.partition_id_tensor.name
                      if nc.partition_id_tensor else None)
    out_names = r["out_names"]
    out_avals = r["out_avals"]
    in_names = r["in_names"]
    n_params = r["n_params"]
    n_outs = len(out_names)
    all_in_names = list(in_names) + list(out_names)
    if partition_name is not None:
        all_in_names.append(partition_name)

    def _body(*args):
        ins = list(args[:n_params])
        outs_all = []
        for k in range(K):
            zeros_k = list(args[n_params + k * n_outs:
                                n_params + (k + 1) * n_outs])
            operands = ins + zeros_k
            if partition_name is not None:
                operands.append(partition_id_tensor())
            outs = _bass_exec_p.bind(
                *operands,
                out_avals=tuple(out_avals),
                in_names=tuple(all_in_names),
                out_names=tuple(out_names),
                lowering_input_output_aliases=(),
                sim_require_finite=True,
                sim_require_nnan=True,
                nc=nc,
            )
            outs_all.extend(outs)
        return tuple(outs_all)

    mesh = r["mesh"]
    nin = n_params + K * n_outs
    fn = jax.jit(
        shard_map(_body, mesh=mesh,
                  in_specs=(PartitionSpec("core"),) * nin,
                  out_specs=(PartitionSpec("core"),) * (K * n_outs),
                  check_rep=False),
        donate_argnums=tuple(range(n_params, nin)), keep_unused=True)
    return fn
